# revision 21
# baseline (speedup 1.0000x reference)
"""Multi-head attention (B=4, S=2048, D=768, H=12, Dk=64) on 8 trn2 NeuronCores.

Sharding: 8 cores = 4 batches x 2 head-groups (Megatron-style tensor parallel
over heads within a batch).  Core (b, g) computes, for batch b and its 6 heads:
    Qt = (Wq_g/8) @ q[b].T + bq/8   [384, 2048]   (transposed layout, dk on partitions)
    Kt = Wk_g @ k[b].T + bk         [384, 2048]
    V  = v[b] @ Wv_g.T + bv         [2048, 384]   (natural layout, with a ones
                                                   column appended per head)
    per head h, per q-tile:
      St  = Kt_h.T @ Qt_h           scores^T tiles  [k, q]
      Et  = exp(St)                 (max-subtraction skipped; |scores| <~ 6)
      Ut  = [V_h | 1].T @ Et        [65, q]  (row 64 = softmax denominator)
      At  = Ut[0:64] * (1/Ut[64])   attention output^T  [64, q]
    outT_partial = Wo[:, g]^T-proj of At_all   [768, 2048]
Host sums the two head-group partials per batch, transposes, adds bo.

Matmul operands are fp16 (full-rate on the PE, fp32 PSUM accumulation);
score/Ut accumulators and the normalization stay fp32.  The kc loop is
software-pipelined (scores for kc emitted before the A.V matmuls for kc-1)
so the PE never head-of-line blocks on the Scalar engine's exp.
"""

import numpy as np

S = 2048          # sequence length
D = 768           # model dim
HG = 6            # heads per group (per core)
DK = 64           # head dim
GP = HG * DK      # group projection width = 384
P = 128           # partitions
QT = 512          # q-tile (matmul moving free dim)
NQT = S // QT     # 4
NKC = S // P      # 16 key chunks
NDC = D // P      # 6 d-chunks
NMC = GP // P     # 3 dk-chunks (head pairs)
VW = DK + 1       # 65: V columns + ones column

_CACHE = {}


def _ts(t):
    return slice(t * QT, (t + 1) * QT)


def _build_module(debug=False):
    import concourse.mybir as mybir
    import concourse.tile as tile
    from concourse import bacc

    fp32 = mybir.dt.float32
    fh = mybir.dt.float16
    EXP = mybir.ActivationFunctionType.Exp

    nc = bacc.Bacc("TRN2", target_bir_lowering=False, debug=False, num_devices=8)

    xqT_d = nc.dram_tensor("xqT", [NQT, P, NDC, QT], fh, kind="ExternalInput")
    xkT_d = nc.dram_tensor("xkT", [NQT, P, NDC, QT], fh, kind="ExternalInput")
    xvT_d = nc.dram_tensor("xvT", [NQT, P, NDC, QT], fh, kind="ExternalInput")
    wqT_d = nc.dram_tensor("wqT", [P, NDC, GP], fh, kind="ExternalInput")
    wkT_d = nc.dram_tensor("wkT", [P, NDC, GP], fh, kind="ExternalInput")
    wvT_d = nc.dram_tensor("wvT", [P, NDC, GP], fh, kind="ExternalInput")
    woT_d = nc.dram_tensor("woT", [P, NMC, D], fh, kind="ExternalInput")
    bqp_d = nc.dram_tensor("bqp", [P, NMC], fp32, kind="ExternalInput")
    bkp_d = nc.dram_tensor("bkp", [P, NMC], fp32, kind="ExternalInput")
    bv_d = nc.dram_tensor("bv", [1, GP], fh, kind="ExternalInput")
    ones_d = nc.dram_tensor("ones_c", [1, P], fh, kind="ExternalInput")
    sel_d = nc.dram_tensor("sel", [HG, HG * DK], fh, kind="ExternalInput")
    outT_d = nc.dram_tensor("outT", [D, S], fp32, kind="ExternalOutput")
    if debug:
        dbg_qt = nc.dram_tensor("dbg_qt", [P, NMC, S], fh, kind="ExternalOutput")
        dbg_kt = nc.dram_tensor("dbg_kt", [P, NMC, S], fh, kind="ExternalOutput")
        dbg_v = nc.dram_tensor("dbg_v", [P, NKC, HG * VW], fh, kind="ExternalOutput")
        dbg_at = nc.dram_tensor("dbg_at", [P, NMC, QT], fh, kind="ExternalOutput")
        dbg_et = nc.dram_tensor("dbg_et", [P, 2 * QT], fh, kind="ExternalOutput")
        dbg_ut = nc.dram_tensor("dbg_ut", [VW, HG, QT], fh, kind="ExternalOutput")
        dbg_rc = nc.dram_tensor("dbg_rc", [HG, QT], fp32, kind="ExternalOutput")

    with tile.TileContext(nc) as tc:
        with (
            tc.tile_pool(name="persist", bufs=1) as persist,
            tc.tile_pool(name="etp", bufs=6) as etp,
            tc.tile_pool(name="small", bufs=2) as small,
            tc.tile_pool(name="tmpp", bufs=2) as tmpp,
            tc.tile_pool(name="outp", bufs=3) as outp,
            tc.tile_pool(name="atp", bufs=2) as atp,
            tc.tile_pool(name="utcp", bufs=2) as utcp,
            tc.tile_pool(name="stp", bufs=2, space="PSUM") as stp,
            tc.tile_pool(name="utp", bufs=2, space="PSUM") as utp,
            tc.tile_pool(name="pop", bufs=2, space="PSUM") as pop,
        ):
            # ---- constants & weights -------------------------------------
            ones = persist.tile([1, P], fh)
            nc.sync.dma_start(ones, ones_d[:])
            wq_sb = persist.tile([P, NDC, GP], fh)
            nc.sync.dma_start(wq_sb, wqT_d[:])
            wk_sb = persist.tile([P, NDC, GP], fh)
            nc.sync.dma_start(wk_sb, wkT_d[:])
            wv_sb = persist.tile([P, NDC, GP], fh)
            nc.sync.dma_start(wv_sb, wvT_d[:])
            bqp_sb = persist.tile([P, NMC], fp32)
            nc.sync.dma_start(bqp_sb, bqp_d[:])
            bkp_sb = persist.tile([P, NMC], fp32)
            nc.sync.dma_start(bkp_sb, bkp_d[:])
            bv_sb = persist.tile([1, GP], fh)
            nc.sync.dma_start(bv_sb, bv_d[:])

            # input activations, staged once in SBUF (DMA'd in priority
            # order: xk first, then xv interleaved, then xq)
            xk_buf = [
                persist.tile([P, NDC, QT], fh, tag=f"xk{t}", name=f"xk{t}")
                for t in range(NQT)
            ]
            xq_buf = [
                persist.tile([P, NDC, QT], fh, tag=f"xq{t}", name=f"xq{t}")
                for t in range(NQT)
            ]
            xv_buf = [
                persist.tile([P, NDC, QT], fh, tag=f"xv{t}", name=f"xv{t}")
                for t in range(NQT)
            ]

            def stage_x(buf, xd, t, eng):
                eng.dma_start(buf[t], xd[t])

            stage_x(xk_buf, xkT_d, 0, nc.sync)
            stage_x(xq_buf, xqT_d, 0, nc.sync)
            stage_x(xv_buf, xvT_d, 0, nc.sync)
            for t in range(1, NQT):
                stage_x(xk_buf, xkT_d, t, nc.sync)
                stage_x(xv_buf, xvT_d, t, nc.sync)
                stage_x(xq_buf, xqT_d, t, nc.sync)

            # V (one tile per key chunk), Kt (per dk-chunk), Qt (per chunk,
            # q-tile) — fine-grained tiles give the scheduler exact deps.
            v_tiles = [
                persist.tile([P, HG * VW], fh, tag=f"v{si}", name=f"v{si}")
                for si in range(NKC)
            ]
            kt_tiles = [
                [
                    persist.tile([P, QT], fh, tag=f"kt{m}_{t}", name=f"kt{m}_{t}")
                    for t in range(NQT)
                ]
                for m in range(NMC)
            ]
            qt_tiles = [
                [
                    persist.tile([P, QT], fh, tag=f"qt{m}_{t}", name=f"qt{m}_{t}")
                    for t in range(NQT)
                ]
                for m in range(NMC)
            ]

            def proj_qk(buf, w_sb, b_sb, m, t, dst):
                x_t = buf[t]
                ms = slice(m * P, (m + 1) * P)
                ps = pop.tile([P, QT], fp32, tag="po", name="ps_qk")
                for c in range(NDC):
                    nc.tensor.matmul(
                        ps, lhsT=w_sb[:, c, ms], rhs=x_t[:, c, :],
                        start=(c == 0), stop=(c == NDC - 1),
                    )
                nc.vector.tensor_scalar_add(
                    out=dst, in0=ps, scalar1=b_sb[:, m : m + 1]
                )

            def proj_v(si):
                vt = v_tiles[si]
                nc.vector.memset(
                    vt[:].rearrange("p (h e) -> p h e", e=VW)[:, :, DK:VW], 1.0
                )
                xv_t = xv_buf[si // 4][:, :, (si % 4) * P : (si % 4 + 1) * P]
                ps = pop.tile([P, QT], fp32, tag="po", name="ps_v")[:, :GP]
                nc.tensor.matmul(
                    ps, lhsT=ones[0:1, 0:P], rhs=bv_sb[0:1, :],
                    start=True, stop=False,
                )
                for c in range(NDC):
                    nc.tensor.matmul(
                        ps, lhsT=xv_t[:, c], rhs=wv_sb[:, c, :],
                        start=False, stop=(c == NDC - 1),
                    )
                nc.vector.tensor_copy(
                    out=vt[:].rearrange("p (h e) -> p h e", e=VW)[:, :, 0:DK],
                    in_=ps.rearrange("p (h d) -> p h d", d=DK),
                )

            def attention_pair(qt, p, utc, fill=None):
                """Scores+exp+A.V for head pair p over q-tile qt (skewed)."""
                qs = _ts(qt)
                ut0 = utp.tile([VW, QT], fp32, tag="ut", name="ut0")
                ut1 = utp.tile([VW, QT], fp32, tag="ut", name="ut1")
                pend = []  # (et, kc) awaiting their A.V matmuls (skew 2)
                for kc in range(NKC):
                    for fn in (fill or {}).get(kc, []):
                        fn()
                    st = stp.tile([P, 2 * QT], fp32, tag="st", name="st")
                    # two heads packed into PE row groups 0-63 / 64-127
                    ktt = kt_tiles[p][kc // 4]
                    kss = slice((kc % 4) * P, (kc % 4 + 1) * P)
                    nc.tensor.matmul(
                        st[:, 0:QT],
                        lhsT=ktt[0:DK, kss],
                        rhs=qt_tiles[p][qt][0:DK, :],
                        start=True, stop=True,
                    )
                    nc.tensor.matmul(
                        st[:, QT:],
                        lhsT=ktt[DK:P, kss],
                        rhs=qt_tiles[p][qt][DK:P, :],
                        start=True, stop=True,
                    )
                    et = etp.tile([P, 2 * QT], fh, tag="et", name="et")
                    nc.scalar.activation(out=et, in_=st, func=EXP)
                    if debug and qt == 0 and p == 0 and kc == 0:
                        nc.sync.dma_start(out=dbg_et[:], in_=et[:])
                    pend.append((et, kc))
                    if len(pend) > 2:
                        e = pend.pop(0)
                        _av(e[0], e[1], p, ut0, ut1)
                for e in pend:
                    _av(e[0], e[1], p, ut0, ut1)
                # park Ut in SBUF to release the PSUM accumulators
                nc.vector.tensor_copy(out=utc[:, 2 * p, :], in_=ut0)
                nc.vector.tensor_copy(out=utc[:, 2 * p + 1, :], in_=ut1)

            def _av(et, kc, p, ut0, ut1):
                nc.tensor.matmul(
                    ut0,
                    lhsT=v_tiles[kc][:, 2 * VW * p : 2 * VW * p + VW],
                    rhs=et[:, 0:QT],
                    start=(kc == 0), stop=(kc == NKC - 1),
                )
                nc.tensor.matmul(
                    ut1,
                    lhsT=v_tiles[kc][:, 2 * VW * p + VW : 2 * VW * (p + 1)],
                    rhs=et[:, QT:],
                    start=(kc == 0), stop=(kc == NKC - 1),
                )

            tail_rcr = {}

            def tail_recip(qt, utc):
                def f():
                    dn = small.tile([HG, QT], fh, tag="dn", name="dn")
                    for j in range(HG):
                        nc.sync.dma_start(
                            out=dn[j : j + 1, :], in_=utc[DK:VW, j, :]
                        )
                    rc = small.tile([HG, QT], fp32, tag="rc", name="rc")
                    nc.vector.reciprocal(out=rc, in_=dn)
                    rcr = small.tile([HG, QT], fh, tag="rcr", name="rcr")
                    nc.vector.tensor_copy(out=rcr, in_=rc)
                    tail_rcr[qt] = rcr
                return f

            def tail_norm(qt, p, utc, at_t):
                def f():
                    rcr = tail_rcr[qt]
                    bcE = pop.tile([P, QT], fp32, tag="po", name="bcE")[0:DK, :]
                    nc.tensor.matmul(
                        bcE, lhsT=sel_sb[:, DK * 2 * p : DK * (2 * p + 1)],
                        rhs=rcr, start=True, stop=True,
                    )
                    bcO = pop.tile([P, QT], fp32, tag="po", name="bcO")[0:DK, :]
                    nc.tensor.matmul(
                        bcO, lhsT=sel_sb[:, DK * (2 * p + 1) : DK * (2 * p + 2)],
                        rhs=rcr, start=True, stop=True,
                    )
                    nc.vector.tensor_mul(
                        out=at_t[0:DK, p, :], in0=utc[0:DK, 2 * p, :], in1=bcE
                    )
                    sh = tmpp.tile([DK, QT], fh, tag="sh", name="sh")
                    nc.vector.tensor_mul(
                        out=sh, in0=utc[0:DK, 2 * p + 1, :], in1=bcO
                    )
                    nc.sync.dma_start(out=at_t[DK:P, p, :], in_=sh)
                return f

            def tail_out(qt, oc, at_t):
                def f():
                    qs = _ts(qt)
                    os_ = slice(oc * P, (oc + 1) * P)
                    po = pop.tile([P, QT], fp32, tag="po", name="po")
                    for c in range(NMC):
                        nc.tensor.matmul(
                            po, lhsT=wo_sb[:, c, os_], rhs=at_t[:, c, :],
                            start=(c == 0), stop=(c == NMC - 1),
                        )
                    ot = outp.tile([P, QT], fp32, tag="ot", name="ot")
                    nc.vector.tensor_copy(out=ot, in_=po)
                    nc.sync.dma_start(out=outT_d[os_, qs], in_=ot)
                return f

            # ---- emission order: minimal prereqs, then attention with
            # projection work interleaved into the kc loops so the Scalar
            # engine's exp stream (the bottleneck) never starves.
            warm = small.tile([1, 16], fh, tag="warm", name="warm")
            nc.scalar.activation(out=warm, in_=ones[0:1, 0:16], func=EXP)

            proj_qk(xk_buf, wk_sb, bkp_sb, 0, 0, kt_tiles[0][0][:])
            proj_qk(xq_buf, wq_sb, bqp_sb, 0, 0, qt_tiles[0][0][:])
            proj_v(0)
            proj_v(1)
            proj_qk(xk_buf, wk_sb, bkp_sb, 0, 1, kt_tiles[0][1][:])

            def kfill(m, t):
                return lambda: proj_qk(
                    xk_buf, wk_sb, bkp_sb, m, t, kt_tiles[m][t][:]
                )

            def qfill(m, t):
                return lambda: proj_qk(
                    xq_buf, wq_sb, bqp_sb, m, t, qt_tiles[m][t][:]
                )

            from collections import defaultdict

            fills = defaultdict(lambda: defaultdict(list))
            for si in range(2, NKC):
                fills[(0, 0)][si - 2].append(lambda si=si: proj_v(si))
            fills[(0, 0)][6].append(kfill(0, 2))
            fills[(0, 0)][10].append(kfill(0, 3))
            fills[(0, 0)][14].append(kfill(1, 0))
            fills[(0, 0)][15].append(qfill(1, 0))
            fills[(0, 1)][2].append(kfill(1, 1))
            fills[(0, 1)][6].append(kfill(1, 2))
            fills[(0, 1)][10].append(kfill(1, 3))
            fills[(0, 1)][13].append(kfill(2, 0))
            fills[(0, 1)][15].append(qfill(2, 0))
            fills[(0, 2)][2].append(kfill(2, 1))
            fills[(0, 2)][6].append(kfill(2, 2))
            fills[(0, 2)][10].append(kfill(2, 3))
            fills[(0, 2)][13].append(qfill(0, 1))
            qlate = [(1, 1), (2, 1), (0, 2), (1, 2), (2, 2), (0, 3), (1, 3), (2, 3)]
            for i, (m, t) in enumerate(qlate):
                qt_i, p_i = 1 + i // 3, i % 3
                fills[(qt_i, p_i)][6].append(qfill(m, t))

            wo_sb = persist.tile([P, NMC, D], fh)
            nc.sync.dma_start(wo_sb, woT_d[:])
            sel_sb = persist.tile([HG, HG * DK], fh)
            nc.sync.dma_start(sel_sb, sel_d[:])

            def last_pair_tail(p, utc, at_t):
                """Per-pair normalize for the final q-tile (overlaps its own
                later pairs instead of a nonexistent next q-tile)."""
                def f():
                    dn = small.tile([2, QT], fh, tag="dn2", name="dn2")
                    nc.sync.dma_start(out=dn[0:1, :], in_=utc[DK:VW, 2 * p, :])
                    nc.sync.dma_start(out=dn[1:2, :], in_=utc[DK:VW, 2 * p + 1, :])
                    rc = small.tile([2, QT], fp32, tag="rc2", name="rc2")
                    nc.vector.reciprocal(out=rc, in_=dn)
                    rcr = small.tile([2, QT], fh, tag="rcr2", name="rcr2")
                    nc.vector.tensor_copy(out=rcr, in_=rc)
                    bcE = pop.tile([P, QT], fp32, tag="po", name="bcE")[0:DK, :]
                    nc.tensor.matmul(
                        bcE, lhsT=sel_sb[0:2, 0:DK], rhs=rcr,
                        start=True, stop=True,
                    )
                    bcO = pop.tile([P, QT], fp32, tag="po", name="bcO")[0:DK, :]
                    nc.tensor.matmul(
                        bcO, lhsT=sel_sb[0:2, DK:P], rhs=rcr,
                        start=True, stop=True,
                    )
                    nc.vector.tensor_mul(
                        out=at_t[0:DK, p, :], in0=utc[0:DK, 2 * p, :], in1=bcE
                    )
                    sh = tmpp.tile([DK, QT], fh, tag="sh", name="sh")
                    nc.vector.tensor_mul(
                        out=sh, in0=utc[0:DK, 2 * p + 1, :], in1=bcO
                    )
                    nc.sync.dma_start(out=at_t[DK:P, p, :], in_=sh)
                return f

            prev = None  # (qt, utc, at_t) awaiting normalization + outproj
            last = NQT - 1
            for qt in range(NQT):
                at_t = atp.tile([P, NMC, QT], fh, tag="at", name="at_t")
                utc = utcp.tile([VW, HG, QT], fh, tag="utc", name="utc")
                for p in range(NMC):
                    f = dict(fills.get((qt, p), {}))
                    if prev is not None:
                        pq, putc, pat = prev
                        if p == 0:
                            f.setdefault(2, []).append(tail_recip(pq, putc))
                            f.setdefault(8, []).append(tail_norm(pq, 0, putc, pat))
                        elif p == 1:
                            f.setdefault(2, []).append(tail_norm(pq, 1, putc, pat))
                            f.setdefault(8, []).append(tail_norm(pq, 2, putc, pat))
                        else:
                            f.setdefault(2, []).append(tail_out(pq, 0, pat))
                            f.setdefault(5, []).append(tail_out(pq, 1, pat))
                            f.setdefault(8, []).append(tail_out(pq, 2, pat))
                            f.setdefault(11, []).append(tail_out(pq, 3, pat))
                            f.setdefault(13, []).append(tail_out(pq, 4, pat))
                            f.setdefault(15, []).append(tail_out(pq, 5, pat))
                    if qt == last and p > 0:
                        f.setdefault(3, []).append(
                            last_pair_tail(p - 1, utc, at_t)
                        )
                    attention_pair(qt, p, utc, f)
                prev = (qt, utc, at_t)
            pq, putc, pat = prev
            last_pair_tail(NMC - 1, putc, pat)()
            for oc in range(NDC):
                tail_out(pq, oc, pat)()

            if debug:
                for m in range(NMC):
                    for t in range(NQT):
                        nc.sync.dma_start(
                            out=dbg_kt[:, m, _ts(t)], in_=kt_tiles[m][t][:]
                        )
                        nc.sync.dma_start(
                            out=dbg_qt[:, m, _ts(t)], in_=qt_tiles[m][t][:]
                        )
                for si in range(NKC):
                    nc.sync.dma_start(out=dbg_v[:, si, :], in_=v_tiles[si][:])
    nc.compile()
    return nc


def _get_module(debug=False):
    key = ("nc", debug)
    if key not in _CACHE:
        _CACHE[key] = _build_module(debug)
    return _CACHE[key]


def _xs(x):
    """[S, D] activations -> staged [NQT, P, NDC, QT] fp16 of x.T."""
    xT = np.ascontiguousarray(x.T)  # [D, S]
    return np.ascontiguousarray(
        xT.reshape(NDC, P, NQT, QT).transpose(2, 1, 0, 3)
    ).astype(np.float16)


def _ws(wT):
    """[D, GP] weight -> [P, NDC, GP] fp16."""
    return np.ascontiguousarray(
        wT.reshape(NDC, P, GP).transpose(1, 0, 2)
    ).astype(np.float16)


def _wos(woT):
    """[GP, D] -> [P, NMC, D] fp16."""
    return np.ascontiguousarray(
        woT.reshape(NMC, P, D).transpose(1, 0, 2)
    ).astype(np.float16)


def make_in_maps(q, k, v, Wq, bq, Wk, bk, Wv, bv, Wo, bo):
    """Shard the full inputs into the 8 per-core input maps."""
    f32, f16 = np.float32, np.float16
    q = np.asarray(q, f32)
    k = np.asarray(k, f32)
    v = np.asarray(v, f32)
    Wq = np.asarray(Wq, f32)
    Wk = np.asarray(Wk, f32)
    Wv = np.asarray(Wv, f32)
    Wo = np.asarray(Wo, f32)
    bq = np.asarray(bq, f32)
    bk = np.asarray(bk, f32)
    bv = np.asarray(bv, f32)
    B = q.shape[0]
    scale = f32(1.0 / np.sqrt(DK))
    # sel: kron(I6, ones(1,64)) -- column block j broadcasts head j's row
    sel = np.kron(np.eye(HG, dtype=f16), np.ones((1, DK), f16))
    in_maps = []
    for core in range(2 * B):
        b, g = core // 2, core % 2
        hs = slice(GP * g, GP * (g + 1))
        in_maps.append(
            {
                "xqT": _xs(q[b]),
                "xkT": _xs(k[b]),
                "xvT": _xs(v[b]),
                "wqT": _ws((Wq[hs, :] * scale).T),
                "wkT": _ws(Wk[hs, :].T),
                "wvT": _ws(Wv[hs, :].T),
                "woT": _wos(Wo[:, hs].T),
                "bqp": np.ascontiguousarray(
                    (bq[hs] * scale).reshape(NMC, P).T
                ).astype(f32),
                "bkp": np.ascontiguousarray(bk[hs].reshape(NMC, P).T).astype(f32),
                "bv": bv[hs].reshape(1, GP).astype(f16),
                "ones_c": np.ones((1, P), f16),
                "sel": sel,
            }
        )
    return in_maps


def gather_output(results, bo, B=4):
    bo = np.asarray(bo, np.float32)
    out = np.empty((B, S, D), np.float32)
    for b in range(B):
        acc = results[2 * b]["outT"] + results[2 * b + 1]["outT"]
        out[b] = acc.T + bo
    return out


def run(inputs, trace=False, debug=False):
    """Run the kernel; returns (output, BassKernelResults)."""
    import concourse.bass_utils as bass_utils

    nc = _get_module(debug)
    in_maps = make_in_maps(**inputs)
    res = bass_utils.run_bass_kernel_spmd(
        nc, in_maps, core_ids=list(range(8)), trace=trace,
        trace_cores=[0] if trace else None,
    )
    out = gather_output(res.results, inputs["bo"])
    return out, res


def kernel(**inputs) -> np.ndarray:
    out, _ = run(inputs, trace=False)
    return out


# revision 22
# speedup vs baseline: 1.0197x; 1.0197x over previous
"""Multi-head attention (B=4, S=2048, D=768, H=12, Dk=64) on 8 trn2 NeuronCores.

Sharding: 8 cores = 4 batches x 2 head-groups (Megatron-style tensor parallel
over heads within a batch).  Core (b, g) computes, for batch b and its 6 heads:
    Qt = (Wq_g/8) @ q[b].T + bq/8   [384, 2048]   (transposed layout, dk on partitions)
    Kt = Wk_g @ k[b].T + bk         [384, 2048]
    V  = v[b] @ Wv_g.T + bv         [2048, 384]   (natural layout, with a ones
                                                   column appended per head)
    per head h, per q-tile:
      St  = Kt_h.T @ Qt_h           scores^T tiles  [k, q]
      Et  = exp(St)                 (max-subtraction skipped; |scores| <~ 6)
      Ut  = [V_h | 1].T @ Et        [65, q]  (row 64 = softmax denominator)
      At  = Ut[0:64] * (1/Ut[64])   attention output^T  [64, q]
    outT_partial = Wo[:, g]^T-proj of At_all   [768, 2048]
Host sums the two head-group partials per batch, transposes, adds bo.

Matmul operands are fp16 (full-rate on the PE, fp32 PSUM accumulation);
score/Ut accumulators and the normalization stay fp32.  The kc loop is
software-pipelined (scores for kc emitted before the A.V matmuls for kc-1)
so the PE never head-of-line blocks on the Scalar engine's exp.
"""

import numpy as np

S = 2048          # sequence length
D = 768           # model dim
HG = 6            # heads per group (per core)
DK = 64           # head dim
GP = HG * DK      # group projection width = 384
P = 128           # partitions
QT = 512          # q-tile (matmul moving free dim)
NQT = S // QT     # 4
NKC = S // P      # 16 key chunks
NDC = D // P      # 6 d-chunks
NMC = GP // P     # 3 dk-chunks (head pairs)
VW = DK + 1       # 65: V columns + ones column

_CACHE = {}


def _ts(t):
    return slice(t * QT, (t + 1) * QT)


def _build_module(debug=False):
    import concourse.mybir as mybir
    import concourse.tile as tile
    from concourse import bacc

    fp32 = mybir.dt.float32
    fh = mybir.dt.float16
    EXP = mybir.ActivationFunctionType.Exp

    nc = bacc.Bacc("TRN2", target_bir_lowering=False, debug=False, num_devices=8)

    xqT_d = nc.dram_tensor("xqT", [NQT, P, NDC, QT], fh, kind="ExternalInput")
    xkT_d = nc.dram_tensor("xkT", [NQT, P, NDC, QT], fh, kind="ExternalInput")
    xvT_d = nc.dram_tensor("xvT", [NQT, P, NDC, QT], fh, kind="ExternalInput")
    wqT_d = nc.dram_tensor("wqT", [P, NDC, GP], fh, kind="ExternalInput")
    wkT_d = nc.dram_tensor("wkT", [P, NDC, GP], fh, kind="ExternalInput")
    wvT_d = nc.dram_tensor("wvT", [P, NDC, GP], fh, kind="ExternalInput")
    woT_d = nc.dram_tensor("woT", [P, NMC, D], fh, kind="ExternalInput")
    bqp_d = nc.dram_tensor("bqp", [P, NMC], fp32, kind="ExternalInput")
    bkp_d = nc.dram_tensor("bkp", [P, NMC], fp32, kind="ExternalInput")
    bv_d = nc.dram_tensor("bv", [1, GP], fh, kind="ExternalInput")
    ones_d = nc.dram_tensor("ones_c", [1, P], fh, kind="ExternalInput")
    sel_d = nc.dram_tensor("sel", [HG, HG * DK], fh, kind="ExternalInput")
    outT_d = nc.dram_tensor("outT", [D, S], fp32, kind="ExternalOutput")
    if debug:
        dbg_qt = nc.dram_tensor("dbg_qt", [P, NMC, S], fh, kind="ExternalOutput")
        dbg_kt = nc.dram_tensor("dbg_kt", [P, NMC, S], fh, kind="ExternalOutput")
        dbg_v = nc.dram_tensor("dbg_v", [P, NKC, HG * VW], fh, kind="ExternalOutput")
        dbg_at = nc.dram_tensor("dbg_at", [P, NMC, QT], fh, kind="ExternalOutput")
        dbg_et = nc.dram_tensor("dbg_et", [P, 2 * QT], fh, kind="ExternalOutput")
        dbg_ut = nc.dram_tensor("dbg_ut", [VW, HG, QT], fh, kind="ExternalOutput")
        dbg_rc = nc.dram_tensor("dbg_rc", [HG, QT], fp32, kind="ExternalOutput")

    with tile.TileContext(nc) as tc:
        with (
            tc.tile_pool(name="persist", bufs=1) as persist,
            tc.tile_pool(name="etp", bufs=6) as etp,
            tc.tile_pool(name="small", bufs=2) as small,
            tc.tile_pool(name="tmpp", bufs=2) as tmpp,
            tc.tile_pool(name="outp", bufs=3) as outp,
            tc.tile_pool(name="atp", bufs=2) as atp,
            tc.tile_pool(name="utcp", bufs=2) as utcp,
            tc.tile_pool(name="stp", bufs=2, space="PSUM") as stp,
            tc.tile_pool(name="utp", bufs=2, space="PSUM") as utp,
            tc.tile_pool(name="pop", bufs=2, space="PSUM") as pop,
        ):
            # ---- constants & weights -------------------------------------
            ones = persist.tile([1, P], fh)
            nc.sync.dma_start(ones, ones_d[:])
            wq_sb = persist.tile([P, NDC, GP], fh)
            nc.sync.dma_start(wq_sb, wqT_d[:])
            wk_sb = persist.tile([P, NDC, GP], fh)
            nc.sync.dma_start(wk_sb, wkT_d[:])
            wv_sb = persist.tile([P, NDC, GP], fh)
            nc.sync.dma_start(wv_sb, wvT_d[:])
            bqp_sb = persist.tile([P, NMC], fp32)
            nc.sync.dma_start(bqp_sb, bqp_d[:])
            bkp_sb = persist.tile([P, NMC], fp32)
            nc.sync.dma_start(bkp_sb, bkp_d[:])
            bv_sb = persist.tile([1, GP], fh)
            nc.sync.dma_start(bv_sb, bv_d[:])

            # input activations, staged once in SBUF (DMA'd in priority
            # order: xk first, then xv interleaved, then xq)
            xk_buf = [
                persist.tile([P, NDC, QT], fh, tag=f"xk{t}", name=f"xk{t}")
                for t in range(NQT)
            ]
            xq_buf = [
                persist.tile([P, NDC, QT], fh, tag=f"xq{t}", name=f"xq{t}")
                for t in range(NQT)
            ]
            xv_buf = [
                persist.tile([P, NDC, QT], fh, tag=f"xv{t}", name=f"xv{t}")
                for t in range(NQT)
            ]

            def stage_x(buf, xd, t, eng):
                eng.dma_start(buf[t], xd[t])

            stage_x(xk_buf, xkT_d, 0, nc.sync)
            stage_x(xq_buf, xqT_d, 0, nc.sync)
            stage_x(xv_buf, xvT_d, 0, nc.sync)
            for t in range(1, NQT):
                stage_x(xk_buf, xkT_d, t, nc.sync)
                stage_x(xv_buf, xvT_d, t, nc.sync)
                stage_x(xq_buf, xqT_d, t, nc.sync)

            # V (one tile per key chunk), Kt (per dk-chunk), Qt (per chunk,
            # q-tile) — fine-grained tiles give the scheduler exact deps.
            v_tiles = [
                persist.tile([P, HG * VW], fh, tag=f"v{si}", name=f"v{si}")
                for si in range(NKC)
            ]
            kt_tiles = [
                [
                    persist.tile([P, QT], fh, tag=f"kt{m}_{t}", name=f"kt{m}_{t}")
                    for t in range(NQT)
                ]
                for m in range(NMC)
            ]
            qt_tiles = [
                [
                    persist.tile([P, QT], fh, tag=f"qt{m}_{t}", name=f"qt{m}_{t}")
                    for t in range(NQT)
                ]
                for m in range(NMC)
            ]

            def proj_qk(buf, w_sb, b_sb, m, t, dst):
                x_t = buf[t]
                ms = slice(m * P, (m + 1) * P)
                ps = pop.tile([P, QT], fp32, tag="po", name="ps_qk")
                for c in range(NDC):
                    nc.tensor.matmul(
                        ps, lhsT=w_sb[:, c, ms], rhs=x_t[:, c, :],
                        start=(c == 0), stop=(c == NDC - 1),
                    )
                nc.vector.tensor_scalar_add(
                    out=dst, in0=ps, scalar1=b_sb[:, m : m + 1]
                )

            def proj_v(si):
                vt = v_tiles[si]
                nc.vector.memset(
                    vt[:].rearrange("p (h e) -> p h e", e=VW)[:, :, DK:VW], 1.0
                )
                xv_t = xv_buf[si // 4][:, :, (si % 4) * P : (si % 4 + 1) * P]
                ps = pop.tile([P, QT], fp32, tag="po", name="ps_v")[:, :GP]
                nc.tensor.matmul(
                    ps, lhsT=ones[0:1, 0:P], rhs=bv_sb[0:1, :],
                    start=True, stop=False,
                )
                for c in range(NDC):
                    nc.tensor.matmul(
                        ps, lhsT=xv_t[:, c], rhs=wv_sb[:, c, :],
                        start=False, stop=(c == NDC - 1),
                    )
                nc.vector.tensor_copy(
                    out=vt[:].rearrange("p (h e) -> p h e", e=VW)[:, :, 0:DK],
                    in_=ps.rearrange("p (h d) -> p h d", d=DK),
                )

            def attention_pair(qt, p, utc, fill=None):
                """Scores+exp+A.V for head pair p over q-tile qt (skewed)."""
                qs = _ts(qt)
                ut0 = utp.tile([VW, QT], fp32, tag="ut", name="ut0")
                ut1 = utp.tile([VW, QT], fp32, tag="ut", name="ut1")
                pend = []  # (et, kc) awaiting their A.V matmuls (skew 2)
                for kc in range(NKC):
                    for fn in (fill or {}).get(kc, []):
                        fn()
                    st = stp.tile([P, 2 * QT], fp32, tag="st", name="st")
                    # two heads packed into PE row groups 0-63 / 64-127
                    ktt = kt_tiles[p][kc // 4]
                    kss = slice((kc % 4) * P, (kc % 4 + 1) * P)
                    nc.tensor.matmul(
                        st[:, 0:QT],
                        lhsT=ktt[0:DK, kss],
                        rhs=qt_tiles[p][qt][0:DK, :],
                        start=True, stop=True,
                    )
                    nc.tensor.matmul(
                        st[:, QT:],
                        lhsT=ktt[DK:P, kss],
                        rhs=qt_tiles[p][qt][DK:P, :],
                        start=True, stop=True,
                    )
                    et = etp.tile([P, 2 * QT], fh, tag="et", name="et")
                    nc.scalar.activation(out=et, in_=st, func=EXP)
                    if debug and qt == 0 and p == 0 and kc == 0:
                        nc.sync.dma_start(out=dbg_et[:], in_=et[:])
                    pend.append((et, kc))
                    if len(pend) > 2:
                        e = pend.pop(0)
                        _av(e[0], e[1], p, ut0, ut1)
                for e in pend:
                    _av(e[0], e[1], p, ut0, ut1)
                # park Ut in SBUF to release the PSUM accumulators
                nc.vector.tensor_copy(out=utc[:, 2 * p, :], in_=ut0)
                nc.vector.tensor_copy(out=utc[:, 2 * p + 1, :], in_=ut1)

            def _av(et, kc, p, ut0, ut1):
                nc.tensor.matmul(
                    ut0,
                    lhsT=v_tiles[kc][:, 2 * VW * p : 2 * VW * p + VW],
                    rhs=et[:, 0:QT],
                    start=(kc == 0), stop=(kc == NKC - 1),
                )
                nc.tensor.matmul(
                    ut1,
                    lhsT=v_tiles[kc][:, 2 * VW * p + VW : 2 * VW * (p + 1)],
                    rhs=et[:, QT:],
                    start=(kc == 0), stop=(kc == NKC - 1),
                )

            tail_rcr = {}

            def tail_recip(qt, utc):
                def f():
                    dn = small.tile([HG, QT], fh, tag="dn", name="dn")
                    for j in range(HG):
                        nc.sync.dma_start(
                            out=dn[j : j + 1, :], in_=utc[DK:VW, j, :]
                        )
                    rc = small.tile([HG, QT], fp32, tag="rc", name="rc")
                    nc.vector.reciprocal(out=rc, in_=dn)
                    rcr = small.tile([HG, QT], fh, tag="rcr", name="rcr")
                    nc.vector.tensor_copy(out=rcr, in_=rc)
                    tail_rcr[qt] = rcr
                return f

            def tail_norm(qt, p, utc, at_t):
                def f():
                    rcr = tail_rcr[qt]
                    bcE = pop.tile([P, QT], fp32, tag="po", name="bcE")[0:DK, :]
                    nc.tensor.matmul(
                        bcE, lhsT=sel_sb[:, DK * 2 * p : DK * (2 * p + 1)],
                        rhs=rcr, start=True, stop=True,
                    )
                    bcO = pop.tile([P, QT], fp32, tag="po", name="bcO")[0:DK, :]
                    nc.tensor.matmul(
                        bcO, lhsT=sel_sb[:, DK * (2 * p + 1) : DK * (2 * p + 2)],
                        rhs=rcr, start=True, stop=True,
                    )
                    nc.vector.tensor_mul(
                        out=at_t[0:DK, p, :], in0=utc[0:DK, 2 * p, :], in1=bcE
                    )
                    sh = tmpp.tile([DK, QT], fh, tag="sh", name="sh")
                    nc.vector.tensor_mul(
                        out=sh, in0=utc[0:DK, 2 * p + 1, :], in1=bcO
                    )
                    nc.sync.dma_start(out=at_t[DK:P, p, :], in_=sh)
                return f

            def tail_out(qt, oc, at_t, final=False):
                def f():
                    qs = _ts(qt)
                    os_ = slice(oc * P, (oc + 1) * P)
                    if final and oc % 2 == 0:
                        po = stp.tile(
                            [P, 2 * QT], fp32, tag="st", name="po_f"
                        )[:, :QT]
                    else:
                        po = pop.tile([P, QT], fp32, tag="po", name="po")
                    for c in range(NMC):
                        nc.tensor.matmul(
                            po, lhsT=wo_sb[:, c, os_], rhs=at_t[:, c, :],
                            start=(c == 0), stop=(c == NMC - 1),
                        )
                    ot = outp.tile([P, QT], fp32, tag="ot", name="ot")
                    nc.vector.tensor_copy(out=ot, in_=po)
                    nc.sync.dma_start(out=outT_d[os_, qs], in_=ot)
                return f

            # ---- emission order: minimal prereqs, then attention with
            # projection work interleaved into the kc loops so the Scalar
            # engine's exp stream (the bottleneck) never starves.
            warm = small.tile([1, 16], fh, tag="warm", name="warm")
            nc.scalar.activation(out=warm, in_=ones[0:1, 0:16], func=EXP)

            proj_qk(xk_buf, wk_sb, bkp_sb, 0, 0, kt_tiles[0][0][:])
            proj_qk(xq_buf, wq_sb, bqp_sb, 0, 0, qt_tiles[0][0][:])
            proj_v(0)
            proj_v(1)
            proj_qk(xk_buf, wk_sb, bkp_sb, 0, 1, kt_tiles[0][1][:])
            proj_v(2)
            proj_v(3)

            def kfill(m, t):
                return lambda: proj_qk(
                    xk_buf, wk_sb, bkp_sb, m, t, kt_tiles[m][t][:]
                )

            def qfill(m, t):
                return lambda: proj_qk(
                    xq_buf, wq_sb, bqp_sb, m, t, qt_tiles[m][t][:]
                )

            from collections import defaultdict

            fills = defaultdict(lambda: defaultdict(list))
            for si in range(4, NKC):
                fills[(0, 0)][si - 3].append(lambda si=si: proj_v(si))
            fills[(0, 0)][6].append(kfill(0, 2))
            fills[(0, 0)][10].append(kfill(0, 3))
            fills[(0, 0)][14].append(kfill(1, 0))
            fills[(0, 0)][15].append(qfill(1, 0))
            fills[(0, 1)][2].append(kfill(1, 1))
            fills[(0, 1)][6].append(kfill(1, 2))
            fills[(0, 1)][10].append(kfill(1, 3))
            fills[(0, 1)][13].append(kfill(2, 0))
            fills[(0, 1)][15].append(qfill(2, 0))
            fills[(0, 2)][2].append(kfill(2, 1))
            fills[(0, 2)][6].append(kfill(2, 2))
            fills[(0, 2)][10].append(kfill(2, 3))
            fills[(0, 2)][13].append(qfill(0, 1))
            qlate = [(1, 1), (2, 1), (0, 2), (1, 2), (2, 2), (0, 3), (1, 3), (2, 3)]
            for i, (m, t) in enumerate(qlate):
                qt_i, p_i = 1 + i // 3, i % 3
                fills[(qt_i, p_i)][6].append(qfill(m, t))

            wo_sb = persist.tile([P, NMC, D], fh)
            nc.sync.dma_start(wo_sb, woT_d[:])
            sel_sb = persist.tile([HG, HG * DK], fh)
            nc.sync.dma_start(sel_sb, sel_d[:])

            def last_pair_tail(p, utc, at_t):
                """Per-pair normalize for the final q-tile (overlaps its own
                later pairs instead of a nonexistent next q-tile)."""
                def f():
                    dn = small.tile([2, QT], fh, tag="dn2", name="dn2")
                    nc.sync.dma_start(out=dn[0:1, :], in_=utc[DK:VW, 2 * p, :])
                    nc.sync.dma_start(out=dn[1:2, :], in_=utc[DK:VW, 2 * p + 1, :])
                    rc = small.tile([2, QT], fp32, tag="rc2", name="rc2")
                    nc.vector.reciprocal(out=rc, in_=dn)
                    rcr = small.tile([2, QT], fh, tag="rcr2", name="rcr2")
                    nc.vector.tensor_copy(out=rcr, in_=rc)
                    bcE = pop.tile([P, QT], fp32, tag="po", name="bcE")[0:DK, :]
                    nc.tensor.matmul(
                        bcE, lhsT=sel_sb[0:2, 0:DK], rhs=rcr,
                        start=True, stop=True,
                    )
                    bcO = pop.tile([P, QT], fp32, tag="po", name="bcO")[0:DK, :]
                    nc.tensor.matmul(
                        bcO, lhsT=sel_sb[0:2, DK:P], rhs=rcr,
                        start=True, stop=True,
                    )
                    nc.vector.tensor_mul(
                        out=at_t[0:DK, p, :], in0=utc[0:DK, 2 * p, :], in1=bcE
                    )
                    sh = tmpp.tile([DK, QT], fh, tag="sh", name="sh")
                    nc.vector.tensor_mul(
                        out=sh, in0=utc[0:DK, 2 * p + 1, :], in1=bcO
                    )
                    nc.sync.dma_start(out=at_t[DK:P, p, :], in_=sh)
                return f

            prev = None  # (qt, utc, at_t) awaiting normalization + outproj
            last = NQT - 1
            for qt in range(NQT):
                at_t = atp.tile([P, NMC, QT], fh, tag="at", name="at_t")
                utc = utcp.tile([VW, HG, QT], fh, tag="utc", name="utc")
                for p in range(NMC):
                    f = dict(fills.get((qt, p), {}))
                    if prev is not None:
                        pq, putc, pat = prev
                        if p == 0:
                            f.setdefault(2, []).append(tail_recip(pq, putc))
                            f.setdefault(8, []).append(tail_norm(pq, 0, putc, pat))
                        elif p == 1:
                            f.setdefault(2, []).append(tail_norm(pq, 1, putc, pat))
                            f.setdefault(8, []).append(tail_norm(pq, 2, putc, pat))
                        else:
                            f.setdefault(2, []).append(tail_out(pq, 0, pat))
                            f.setdefault(5, []).append(tail_out(pq, 1, pat))
                            f.setdefault(8, []).append(tail_out(pq, 2, pat))
                            f.setdefault(11, []).append(tail_out(pq, 3, pat))
                            f.setdefault(13, []).append(tail_out(pq, 4, pat))
                            f.setdefault(15, []).append(tail_out(pq, 5, pat))
                    if qt == last and p > 0:
                        f.setdefault(3, []).append(
                            last_pair_tail(p - 1, utc, at_t)
                        )
                    attention_pair(qt, p, utc, f)
                prev = (qt, utc, at_t)
            pq, putc, pat = prev
            last_pair_tail(NMC - 1, putc, pat)()
            for oc in range(NDC):
                tail_out(pq, oc, pat, final=True)()

            if debug:
                for m in range(NMC):
                    for t in range(NQT):
                        nc.sync.dma_start(
                            out=dbg_kt[:, m, _ts(t)], in_=kt_tiles[m][t][:]
                        )
                        nc.sync.dma_start(
                            out=dbg_qt[:, m, _ts(t)], in_=qt_tiles[m][t][:]
                        )
                for si in range(NKC):
                    nc.sync.dma_start(out=dbg_v[:, si, :], in_=v_tiles[si][:])
    nc.compile()
    return nc


def _get_module(debug=False):
    key = ("nc", debug)
    if key not in _CACHE:
        _CACHE[key] = _build_module(debug)
    return _CACHE[key]


def _xs(x):
    """[S, D] activations -> staged [NQT, P, NDC, QT] fp16 of x.T."""
    xT = np.ascontiguousarray(x.T)  # [D, S]
    return np.ascontiguousarray(
        xT.reshape(NDC, P, NQT, QT).transpose(2, 1, 0, 3)
    ).astype(np.float16)


def _ws(wT):
    """[D, GP] weight -> [P, NDC, GP] fp16."""
    return np.ascontiguousarray(
        wT.reshape(NDC, P, GP).transpose(1, 0, 2)
    ).astype(np.float16)


def _wos(woT):
    """[GP, D] -> [P, NMC, D] fp16."""
    return np.ascontiguousarray(
        woT.reshape(NMC, P, D).transpose(1, 0, 2)
    ).astype(np.float16)


def make_in_maps(q, k, v, Wq, bq, Wk, bk, Wv, bv, Wo, bo):
    """Shard the full inputs into the 8 per-core input maps."""
    f32, f16 = np.float32, np.float16
    q = np.asarray(q, f32)
    k = np.asarray(k, f32)
    v = np.asarray(v, f32)
    Wq = np.asarray(Wq, f32)
    Wk = np.asarray(Wk, f32)
    Wv = np.asarray(Wv, f32)
    Wo = np.asarray(Wo, f32)
    bq = np.asarray(bq, f32)
    bk = np.asarray(bk, f32)
    bv = np.asarray(bv, f32)
    B = q.shape[0]
    scale = f32(1.0 / np.sqrt(DK))
    # sel: kron(I6, ones(1,64)) -- column block j broadcasts head j's row
    sel = np.kron(np.eye(HG, dtype=f16), np.ones((1, DK), f16))
    in_maps = []
    for core in range(2 * B):
        b, g = core // 2, core % 2
        hs = slice(GP * g, GP * (g + 1))
        in_maps.append(
            {
                "xqT": _xs(q[b]),
                "xkT": _xs(k[b]),
                "xvT": _xs(v[b]),
                "wqT": _ws((Wq[hs, :] * scale).T),
                "wkT": _ws(Wk[hs, :].T),
                "wvT": _ws(Wv[hs, :].T),
                "woT": _wos(Wo[:, hs].T),
                "bqp": np.ascontiguousarray(
                    (bq[hs] * scale).reshape(NMC, P).T
                ).astype(f32),
                "bkp": np.ascontiguousarray(bk[hs].reshape(NMC, P).T).astype(f32),
                "bv": bv[hs].reshape(1, GP).astype(f16),
                "ones_c": np.ones((1, P), f16),
                "sel": sel,
            }
        )
    return in_maps


def gather_output(results, bo, B=4):
    bo = np.asarray(bo, np.float32)
    out = np.empty((B, S, D), np.float32)
    for b in range(B):
        acc = results[2 * b]["outT"] + results[2 * b + 1]["outT"]
        out[b] = acc.T + bo
    return out


def run(inputs, trace=False, debug=False):
    """Run the kernel; returns (output, BassKernelResults)."""
    import concourse.bass_utils as bass_utils

    nc = _get_module(debug)
    in_maps = make_in_maps(**inputs)
    res = bass_utils.run_bass_kernel_spmd(
        nc, in_maps, core_ids=list(range(8)), trace=trace,
        trace_cores=[0] if trace else None,
    )
    out = gather_output(res.results, inputs["bo"])
    return out, res


def kernel(**inputs) -> np.ndarray:
    out, _ = run(inputs, trace=False)
    return out
